# revision 29
# baseline (speedup 1.0000x reference)
"""Sparse multi-head self-attention (sliding window + global columns) on 8
Trainium2 NeuronCores.

Sharding: fully data-parallel over the sequence dimension. Core c produces
output rows [512c, 512c+512). Each core recomputes k/v for a 128-row halo on
each side of its slice plus the 16 global key rows (j % 256 == 0), so no
collectives are needed.

v3: all matmul operands fp16 (PE runs 1 cycle/col vs multi-pass fp32 HIGH
mode). RoPE'd activations round-trip through DRAM so the din-major transpose
is 8 bulk DMA-transposes instead of 56 PE transposes + copies. v is computed
directly in natural [key, hd] orientation (xT slices stationary). Scores for
the two heads of a pair run concurrently in disjoint PE row groups. Softmax
normalization: denominator via an appended ones-column of v, fast-approx
reciprocal on DVE, partition-broadcast on GpSimd. RoPE adds on GpSimd.
"""
import sys

sys.path.insert(0, "/opt/trn_rl_repo")

import numpy as np
import concourse.bass as bass
import concourse.mybir as mybir
from concourse.tile import TileContext

# ---------------------------------------------------------------- constants
B, T, D = 1, 4096, 1024
H, HD = 16, 64
W = 128
GSTRIDE = 256
ROPE_BASE = 10000.0
NCORES = 8
TLOC = T // NCORES            # 512 own rows per core
HALO = 128
NL = TLOC + 2 * HALO          # 768 rows incl. halo
NG = T // GSTRIDE             # 16 global keys
NR = NL + NG                  # 784 rows incl. globals
NT = NL // 128                # 6 local 128-row key tiles
NQB = TLOC // 128             # 4 query blocks per core
NDT = D // 128                # 8 din tiles
SCALE = 1.0 / np.sqrt(HD)

# per-m query-column ranges within the core's 512 own rows
QS = [0, 0, 0, 128, 256, 384]
QW = [128, 256, 384, 384, 256, 128]

F32 = mybir.dt.float32
F16 = mybir.dt.float16
FT = mybir.ActivationFunctionType

_cache = {}


# ------------------------------------------------------- walrus workaround
def _fix_multi_waits(nc):
    """This walrus build encodes at most ONE sem wait per instruction; hoist
    extra waits onto same-engine NoOps inserted just before the owner."""
    count = 0
    for fn in nc.m.functions:
        for bb in fn.blocks:
            old = bb.instructions
            if not any(
                i.sync_info is not None and len(i.sync_info.on_wait or []) > 1
                for i in old
            ):
                continue
            new = []
            for inst in old:
                si = inst.sync_info
                waits = list(si.on_wait) if si is not None and si.on_wait else []
                if len(waits) > 1:
                    for w in waits[:-1]:
                        count += 1
                        new.append(
                            mybir.InstNoOp(
                                name=f"I-waitfix-{count}",
                                engine=inst.engine,
                                bass_nofuse=True,
                                sync_info=mybir.SyncInfo(on_wait=[w], on_update=[]),
                            )
                        )
                    inst.sync_info = mybir.SyncInfo(
                        on_wait=[waits[-1]], on_update=list(si.on_update or [])
                    )
                new.append(inst)
            bb.instructions = new
    return count


def _bcast_mid(ap2d, reps):
    """[P, F] AP -> [P, reps, F] AP broadcasting along a middle free dim."""
    a = [list(x) for x in ap2d.ap]
    return bass.AP(tensor=ap2d.tensor, offset=ap2d.offset,
                   ap=[a[0], [0, reps], a[1]])


# ------------------------------------------------------------ bass program
def build_program(has_vbias, has_obias, debug=False):
    nc = bass.Bass()

    xcs = nc.dram_tensor("xcs", [NR, 2 * D], F16, kind="ExternalInput")
    wts = nc.dram_tensor("wts", [24, 128, D], F16, kind="ExternalInput")
    wosd = nc.dram_tensor("wosd", [16, 128, 512], F16, kind="ExternalInput")
    ball = nc.dram_tensor("ball", [128, 24], F16, kind="ExternalInput")
    bvp = nc.dram_tensor("bvp", [1, D], F16, kind="ExternalInput")
    bo = nc.dram_tensor("bo", [1, D], F16, kind="ExternalInput")
    mloc = nc.dram_tensor("mloc", [128, NT * 384], F16, kind="ExternalInput")
    mgp = nc.dram_tensor("mgp", [48, TLOC], F16, kind="ExternalInput")
    sel8 = nc.dram_tensor("sel8", [8, 4 * 128], F16, kind="ExternalInput")
    rdram = nc.dram_tensor("rdram", [NR, D], F16, kind="Internal")
    out = nc.dram_tensor("out", [TLOC, D], F32, kind="ExternalOutput")

    with TileContext(nc) as tc:
        _build_body(nc, tc, xcs, wts, wosd, ball, bvp, bo, mloc, mgp,
                    sel8, rdram, out, has_vbias, has_obias, debug)
    _fix_multi_waits(nc)
    return nc


def _build_body(nc, tc, xcs, wts, wosd, ball, bvp, bo, mloc, mgp,
                sel8, rdram, out, has_vbias, has_obias, debug=False):
    from contextlib import ExitStack
    ctx = ExitStack()
    with ctx:
        singles = ctx.enter_context(tc.tile_pool(name="singles", bufs=1))
        ppool = ctx.enter_context(tc.tile_pool(name="ppool", bufs=4))
        npool = ctx.enter_context(tc.tile_pool(name="npool", bufs=2))
        opool = ctx.enter_context(tc.tile_pool(name="opool", bufs=2))
        ps_mm = ctx.enter_context(tc.tile_pool(name="ps_mm", bufs=2, space="PSUM"))
        ps_s = ctx.enter_context(tc.tile_pool(name="ps_s", bufs=3, space="PSUM"))
        ps_g = ctx.enter_context(tc.tile_pool(name="ps_g", bufs=1, space="PSUM"))
        ps_o = ctx.enter_context(tc.tile_pool(name="ps_o", bufs=2, space="PSUM"))

        # ---------------- RoPE -> DRAM -> bulk transpose to xT
        # layout: xcs cols = [evens(512) | odds(512) | cos(512) | sin(512)]
        # wait: 2D = 2048 cols: x evens 0:512, x odds 512:1024 are the
        # permuted x; cos/sin shipped expanded per-head-tiled.
        HD2 = D // 2
        with tc.tile_pool(name="ropepool", bufs=2) as rp:
            for i in range(NT + 1):
                p = 128 if i < NT else NG
                r0 = i * 128
                xc = rp.tile([128, 2 * D], F16, tag="xc", name=f"xc{i}")
                nc.sync.dma_start(xc[:p], xcs[r0:r0 + p, :])
                roped = rp.tile([128, D], F16, tag="roped", name=f"roped{i}")
                tmp = rp.tile([128, D], F16, tag="ropetmp", name=f"ropetmp{i}")
                xe = xc[:p, 0:HD2]
                xo = xc[:p, HD2:D]
                cs = xc[:p, D:D + HD2]
                sn = xc[:p, D + HD2:2 * D]
                nc.vector.tensor_mul(tmp[:p, 0:HD2], xe, cs)
                nc.vector.tensor_mul(tmp[:p, HD2:D], xo, sn)
                nc.vector.tensor_sub(roped[:p, 0:HD2], tmp[:p, 0:HD2],
                                     tmp[:p, HD2:D])
                nc.vector.tensor_mul(tmp[:p, 0:HD2], xe, sn)
                nc.vector.tensor_mul(tmp[:p, HD2:D], xo, cs)
                nc.vector.tensor_add(roped[:p, HD2:D], tmp[:p, 0:HD2],
                                     tmp[:p, HD2:D])
                nc.sync.dma_start(rdram[r0:r0 + p, :], roped[:p])

        xT = [singles.tile([128, NR], F16, tag=f"xT{k}", name=f"xT{k}")
              for k in range(NDT)]
        for k in range(NDT):
            nc.sync.dma_start_transpose(xT[k][:], rdram[:, k * 128:(k + 1) * 128])

        # ---------------- constants / weights (issued on the ACT queue so
        # the sync queue serves the rope/transpose critical path)
        ones = singles.tile([1, 128], F16)
        nc.vector.memset(ones[:], 1.0)
        sel8_sb = singles.tile([8, 4, 128], F16)
        nc.scalar.dma_start(sel8_sb[:], sel8[:].rearrange("p (v q) -> p v q", v=4))
        ball_sb = singles.tile([128, 24], F16)
        nc.scalar.dma_start(ball_sb[:], ball[:])
        bvp_sb = singles.tile([1, D], F16)
        nc.scalar.dma_start(bvp_sb[:], bvp[:])
        bo_sb = singles.tile([1, D], F16)
        nc.scalar.dma_start(bo_sb[:], bo[:])
        mask_sb = singles.tile([128, NT, 384], F16)
        nc.scalar.dma_start(mask_sb[:], mloc[:].rearrange("p (m q) -> p m q", m=NT))
        mgp_sb = singles.tile([48, TLOC], F16)
        nc.scalar.dma_start(mgp_sb[:], mgp[:])
        wq_sb = [singles.tile([128, D], F16, tag=f"wq{i}", name=f"wq{i}")
                 for i in range(NDT)]
        wk_sb = [singles.tile([128, D], F16, tag=f"wk{i}", name=f"wk{i}")
                 for i in range(NDT)]
        wv_sb = [singles.tile([128, D], F16, tag=f"wv{i}", name=f"wv{i}")
                 for i in range(NDT)]
        for i in range(NDT):
            nc.scalar.dma_start(wv_sb[i][:], wts[16 + i])
        for i in range(NDT):
            nc.scalar.dma_start(wq_sb[i][:], wts[i])
            nc.scalar.dma_start(wk_sb[i][:], wts[8 + i])
        wos_sb = [singles.tile([128, 512], F16, tag=f"wos{i}", name=f"wos{i}")
                  for i in range(16)]
        for i in range(16):
            nc.scalar.dma_start(wos_sb[i][:], wosd[i])

        # persistent per-pair tensors
        qT = [singles.tile([128, TLOC], F16, tag=f"qT{c}", name=f"qT{c}")
              for c in range(NDT)]
        kT = [singles.tile([128, NR], F16, tag=f"kT{c}", name=f"kT{c}")
              for c in range(NDT)]
        # v natural: per key tile, [keys, hp, head, 64+ones]
        v_sb = [singles.tile([128, NDT, 2, HD + 1], F16, tag=f"v{m}",
                             name=f"v{m}") for m in range(NT)]
        vg_sb = singles.tile([48, NDT, HD + 1], F16)
        for m in range(NT):
            nc.vector.memset(v_sb[m][:, :, :, HD:HD + 1], 1.0)
        nc.vector.memset(vg_sb[:, :, HD:HD + 1], 1.0)
        oT = [singles.tile([128, TLOC], F16, tag=f"oT{k}", name=f"oT{k}")
              for k in range(NDT)]
        denall = [singles.tile([8, TLOC], F32, tag=f"den{b}", name=f"den{b}")
                  for b in range(2)]
        rcp16 = [singles.tile([8, TLOC], F16, tag=f"rcp{b}", name=f"rcp{b}")
                 for b in range(2)]

        # ---------------- v projection, natural orientation, all pairs at
        # once: stationary xT key-block streams 512 dout columns twice
        for m in range(NT):
            for half in range(2):
                pv = ps_mm.tile([128, 512], F32, tag="mm",
                                name=f"pv{m}_{half}")
                for k in range(NDT):
                    nc.tensor.matmul(pv[:], xT[k][:, m * 128:(m + 1) * 128],
                                     wv_sb[k][:, half * 512:(half + 1) * 512],
                                     start=(k == 0), stop=(k == NDT - 1))
                nc.scalar.copy(
                    v_sb[m][:, 4 * half:4 * half + 4, :, 0:HD],
                    pv[:].rearrange("p (a b c) -> p a b c", a=4, b=2))
        for half in range(2):
            pvg = ps_mm.tile([128, 512], F32, tag="mm", name=f"pvg{half}")
            for k in range(NDT):
                nc.tensor.matmul(pvg[0:NG, :], xT[k][:, NL:NR],
                                 wv_sb[k][:, half * 512:(half + 1) * 512],
                                 start=(k == 0), stop=(k == NDT - 1))
            p3 = pvg[0:NG, :].rearrange("p (a c) -> p a c", a=8)
            nc.scalar.copy(vg_sb[0:NG, 4 * half:4 * half + 4, 0:HD],
                           p3[:, 0::2, :])
            vgst = npool.tile([NG, 4, HD], F16, tag="vgst", name=f"vgst{half}")
            nc.scalar.copy(vgst[:], p3[:, 1::2, :])
            nc.sync.dma_start(vg_sb[32:48, 4 * half:4 * half + 4, 0:HD],
                              vgst[:])

        # ---------------- main pipeline: per head-pair hp
        for hp in range(NDT):
            # --- q projection (own 512 rows)
            pq = ps_mm.tile([128, 512], F32, tag="mm", name=f"pq{hp}")
            for k in range(NDT):
                nc.tensor.matmul(pq[:], wq_sb[k][:, hp * 128:(hp + 1) * 128],
                                 xT[k][:, HALO:HALO + TLOC],
                                 start=(k == 0), stop=(k == NDT - 1))
            nc.scalar.add(qT[hp][:], pq[:], ball_sb[:, hp:hp + 1])

            # --- k projection (768 halo rows + 16 globals)
            pk0 = ps_mm.tile([128, 512], F32, tag="mm", name=f"pk0{hp}")
            for k in range(NDT):
                nc.tensor.matmul(pk0[:], wk_sb[k][:, hp * 128:(hp + 1) * 128],
                                 xT[k][:, 0:512],
                                 start=(k == 0), stop=(k == NDT - 1))
            nc.scalar.add(kT[hp][:, 0:512], pk0[:], ball_sb[:, 8 + hp:9 + hp])
            pk1 = ps_mm.tile([128, 512], F32, tag="mm", name=f"pk1{hp}")
            for k in range(NDT):
                nc.tensor.matmul(pk1[:, 0:NR - 512],
                                 wk_sb[k][:, hp * 128:(hp + 1) * 128],
                                 xT[k][:, 512:NR],
                                 start=(k == 0), stop=(k == NDT - 1))
            nc.scalar.add(kT[hp][:, 512:NR], pk1[:, 0:NR - 512],
                          ball_sb[:, 8 + hp:9 + hp])

            # --- attention: scores for both heads of the pair run in
            # disjoint PE row groups; pv interleaved per key tile
            po_a = ps_o.tile([HD + 1, TLOC], F32, tag="o", name=f"poa{hp}")
            po_b = ps_o.tile([HD + 1, TLOC], F32, tag="o", name=f"pob{hp}")
            for m in range(NT):
                w, qs = QW[m], QS[m]
                psc_a = ps_s.tile([128, 384], F32, tag="s", name=f"psa{hp}_{m}")
                psc_b = ps_s.tile([128, 384], F32, tag="s", name=f"psb{hp}_{m}")
                nc.tensor.matmul(psc_a[:, 0:w],
                                 kT[hp][0:64, m * 128:(m + 1) * 128],
                                 qT[hp][0:64, qs:qs + w], start=True, stop=True)
                nc.tensor.matmul(psc_b[:, 0:w],
                                 kT[hp][64:128, m * 128:(m + 1) * 128],
                                 qT[hp][64:128, qs:qs + w], start=True, stop=True)
                nc.vector.tensor_add(psc_a[:, 0:w], psc_a[:, 0:w],
                                     mask_sb[:, m, 0:w])
                nc.vector.tensor_add(psc_b[:, 0:w], psc_b[:, 0:w],
                                     mask_sb[:, m, 0:w])
                ea = ppool.tile([128, 384], F16, tag="pe", name=f"pea{hp}_{m}")
                eb = ppool.tile([128, 384], F16, tag="pe", name=f"peb{hp}_{m}")
                nc.scalar.activation(ea[:, 0:w], psc_a[:, 0:w], FT.Exp)
                nc.scalar.activation(eb[:, 0:w], psc_b[:, 0:w], FT.Exp)
                nc.tensor.matmul(po_a[:, qs:qs + w], v_sb[m][:, hp, 0, :],
                                 ea[:, 0:w], start=(m == 0), stop=False)
                nc.tensor.matmul(po_b[:, qs:qs + w], v_sb[m][:, hp, 1, :],
                                 eb[:, 0:w], start=(m == 0), stop=False)

            psg = ps_g.tile([48, TLOC], F32, tag="g", name=f"psg{hp}")
            nc.tensor.matmul(psg[0:NG, :], kT[hp][0:64, NL:NR],
                             qT[hp][0:64, :], start=True, stop=True)
            nc.tensor.matmul(psg[32:48, :], kT[hp][64:128, NL:NR],
                             qT[hp][64:128, :], start=True, stop=True,
                             skip_group_check=True)
            nc.vector.tensor_add(psg[:], psg[:], mgp_sb[:])
            peg = ppool.tile([48, TLOC], F16, tag="peg", name=f"peg{hp}")
            nc.scalar.activation(peg[:], psg[:], FT.Exp)

            # --- global pv + denominator extraction + unnormalized copy-out
            bat, slot = hp // 4, (hp % 4) * 2
            for j in range(2):
                po = po_a if j == 0 else po_b
                gb = 32 * j
                nc.tensor.matmul(po[:], vg_sb[gb:gb + NG, hp, :],
                                 peg[gb:gb + NG, :], start=False,
                                 stop=(not has_vbias))
                dn = npool.tile([1, TLOC], F32, tag="dn", name=f"dn{hp}_{j}")
                nc.scalar.copy(dn[:], po[HD:HD + 1, :])
                nc.sync.dma_start(denall[bat][slot + j:slot + j + 1, :], dn[:])
                if has_vbias:
                    denh = npool.tile([1, TLOC], F16, tag="denh",
                                      name=f"denh{hp}_{j}")
                    nc.scalar.copy(denh[:], po[HD:HD + 1, :])
                    nc.tensor.matmul(po[0:HD, :],
                                     bvp_sb[:, hp * 128 + 64 * j:
                                            hp * 128 + 64 * j + HD],
                                     denh[:], start=False, stop=True,
                                     skip_group_check=True)
                if j == 0:
                    nc.vector.tensor_copy(oT[hp][0:64, :], po[0:HD, :])
                else:
                    ost = npool.tile([64, TLOC], F16, tag="ost",
                                     name=f"ost{hp}")
                    nc.vector.tensor_copy(ost[:], po[0:HD, :])
                    nc.sync.dma_start(oT[hp][64:128, :], ost[:])

            # batched reciprocal every 4 head-pairs, then in-place normalize
            if hp % 4 == 3:
                nc.vector.reciprocal(denall[bat][:], denall[bat][:])
                nc.vector.tensor_copy(rcp16[bat][:], denall[bat][:])
                for php in range(hp - 3, hp + 1):
                    pbb = ps_g.tile([128, TLOC], F32, tag="g",
                                    name=f"pb{php}")
                    nc.tensor.matmul(pbb[:], sel8_sb[:, php % 4, :],
                                     rcp16[bat][:],
                                     start=True, stop=True)
                    bc = npool.tile([128, TLOC], F16, tag="bc",
                                    name=f"bc{php}")
                    nc.vector.tensor_copy(bc[:], pbb[:])
                    nc.vector.tensor_mul(oT[php][0:64, :],
                                         oT[php][0:64, :], bc[0:64, :])
                    nc.vector.tensor_mul(oT[php][64:128, :],
                                         oT[php][64:128, :], bc[64:128, :])

        if debug:
            dxT = nc.dram_tensor("dxT", [128, NDT, NR], F16,
                                 kind="ExternalOutput")
            dq = nc.dram_tensor("dq", [128, NDT, TLOC], F16,
                                kind="ExternalOutput")
            dk = nc.dram_tensor("dk", [128, NDT, NR], F16,
                                kind="ExternalOutput")
            dv = nc.dram_tensor("dv", [128, NT, NDT, 2, HD + 1], F16,
                                kind="ExternalOutput")
            dvg = nc.dram_tensor("dvg", [48, NDT, HD + 1], F16,
                                 kind="ExternalOutput")
            doT = nc.dram_tensor("doT", [128, NDT, TLOC], F16,
                                 kind="ExternalOutput")
            drc = nc.dram_tensor("drc", [8, 2, TLOC], F32,
                                 kind="ExternalOutput")
            for k in range(NDT):
                nc.sync.dma_start(dxT[:, k, :], xT[k][:])
                nc.sync.dma_start(dq[:, k, :], qT[k][:])
                nc.sync.dma_start(dk[:, k, :], kT[k][:])
                nc.sync.dma_start(doT[:, k, :], oT[k][:])
            for m in range(NT):
                nc.sync.dma_start(dv[:, m], v_sb[m][:])
            nc.sync.dma_start(dvg[:], vg_sb[:])
            for b in range(2):
                nc.sync.dma_start(drc[:, b, :], denall[b][:])

        # ---------------- output projection
        for qb in range(NQB):
            for chp in range(2):
                pout = ps_mm.tile([128, 512], F32, tag="mm",
                                  name=f"pout{qb}_{chp}")
                for k in range(NDT):
                    nc.tensor.matmul(pout[:], oT[k][:, qb * 128:(qb + 1) * 128],
                                     wos_sb[chp * 8 + k][:],
                                     start=(k == 0),
                                     stop=(k == NDT - 1 and not has_obias))
                if has_obias:
                    nc.tensor.matmul(pout[:], ones[:, 0:128],
                                     bo_sb[:, chp * 512:(chp + 1) * 512],
                                     start=False, stop=True)
                so = opool.tile([128, 512], F32, tag="outsb",
                                name=f"so{qb}_{chp}")
                nc.scalar.copy(so[:], pout[:])
                nc.sync.dma_start(
                    out[qb * 128:(qb + 1) * 128, chp * 512:(chp + 1) * 512],
                    so[:])


# ------------------------------------------------------------ host helpers
def _perm():
    # global evens-then-odds layout: col h*32+i = x[h*64+2i],
    # col 512+h*32+i = x[h*64+2i+1]
    p = np.arange(D).reshape(H, 32, 2)
    return np.concatenate([p[:, :, 0].reshape(-1), p[:, :, 1].reshape(-1)])


def _cos_sin(trows):
    """Tables matching the reference's quirky emb[..., ::2] indexing."""
    inv_freq = (1.0 / (ROPE_BASE ** (np.arange(0, HD, 2, dtype=np.float32) / HD))
                ).astype(np.float32)
    pos = trows.astype(np.float32)
    freqs = pos[:, None] * inv_freq[None, :]
    emb = np.concatenate([freqs, freqs], axis=-1)[:, ::2]      # (n, 32)
    return np.cos(emb).astype(np.float32), np.sin(emb).astype(np.float32)


def _allowed(i, j):
    ok = (np.abs(i - j) <= W) | (j % GSTRIDE == 0) | (j == 0)
    return ok & (j >= 0) & (j < T)


NEG = np.float16(-30000.0)


def make_in_maps(x, in_proj_w, in_proj_b, out_w, out_b):
    perm = _perm()
    x2 = np.asarray(x, np.float32).reshape(T, D)[:, perm]
    wp = np.asarray(in_proj_w, np.float32)[:, perm]
    wt_full = np.ascontiguousarray(wp.T).astype(np.float32)     # (D, 3D)
    wt_full[:, 0:D] *= SCALE
    # wts[8*s + i] = wt_full[128i:128(i+1), s*D:(s+1)*D]  (din-chunk major)
    wts = np.ascontiguousarray(np.concatenate(
        [wt_full[:, s:s + D].reshape(NDT, 128, D) for s in (0, D, 2 * D)],
        axis=0)).astype(np.float16)
    b = np.asarray(in_proj_b, np.float32).copy()
    b[0:D] *= SCALE
    ball = np.ascontiguousarray(b.reshape(24, 128).T).astype(np.float16)
    bvp = np.ascontiguousarray(b[2 * D:][None, :]).astype(np.float16)
    wo_full = np.ascontiguousarray(np.asarray(out_w, np.float32).T)  # (din,dout)
    # wos[8*chp + k] = wo_full[128k:128(k+1), 512chp:512(chp+1)]
    wos = np.ascontiguousarray(
        wo_full.reshape(NDT, 128, 2, 512).transpose(2, 0, 1, 3)
        .reshape(16, 128, 512)).astype(np.float16)
    bo = np.ascontiguousarray(np.asarray(out_b, np.float32)[None, :]
                              ).astype(np.float16)

    tg = np.arange(NG) * GSTRIDE
    cg, sg = _cos_sin(tg)
    csg = np.concatenate([cg, sg], axis=1)

    sel8 = np.zeros((8, 4, 128), np.float16)
    for v in range(4):
        sel8[2 * v, v, 0:64] = 1
        sel8[2 * v + 1, v, 64:128] = 1
    sel8 = np.ascontiguousarray(sel8.reshape(8, 512))

    in_maps = []
    for c in range(NCORES):
        t0 = c * TLOC - HALO
        rows = np.arange(t0, t0 + NL)
        valid = (rows >= 0) & (rows < T)
        xcs = np.zeros((NR, 2 * D), np.float32)
        xcs[:NL, 0:D][valid] = x2[rows[valid]]
        cl, sl = _cos_sin(np.clip(rows, 0, T - 1))
        xcs[:NL, D:] = np.concatenate([np.tile(cl, (1, H)),
                                       np.tile(sl, (1, H))], axis=1)
        xcs[NL:, 0:D] = x2[tg]
        xcs[NL:, D:] = np.concatenate([np.tile(csg[:, 0:32], (1, H)),
                                       np.tile(csg[:, 32:64], (1, H))], axis=1)

        ml = np.full((NT, 128, 384), NEG, np.float16)
        for m in range(NT):
            jj = (t0 + m * 128) + np.arange(128)
            ii = c * TLOC + QS[m] + np.arange(QW[m])
            ml[m, :, 0:QW[m]] = np.where(
                _allowed(ii[None, :], jj[:, None]), np.float16(0), NEG)
        mloc = np.ascontiguousarray(ml.transpose(1, 0, 2).reshape(128, NT * 384))
        iq = c * TLOC + np.arange(TLOC)
        qb = iq // 128
        jg = tg[:, None]
        covered = (jg >= 128 * (qb[None, :] - 1)) & (jg < 128 * (qb[None, :] + 2))
        mg1 = np.where(covered, NEG, np.float16(0)).astype(np.float16)
        mgp = np.full((48, TLOC), NEG, np.float16)
        mgp[0:16] = mg1
        mgp[32:48] = mg1

        in_maps.append({
            "xcs": xcs.astype(np.float16), "wts": wts, "wosd": wos,
            "ball": ball, "bvp": bvp, "bo": bo, "mloc": mloc, "mgp": mgp,
            "sel8": sel8,
        })
    return in_maps


def kernel(x, in_proj_w, in_proj_b, out_w, out_b):
    from concourse.bass_utils import run_bass_kernel_spmd

    has_vbias = bool(np.any(np.asarray(in_proj_b)[2 * D:] != 0))
    has_obias = bool(np.any(np.asarray(out_b) != 0))
    key = ("nc", has_vbias, has_obias)
    if key not in _cache:
        _cache[key] = build_program(has_vbias, has_obias)
        _cache["nc"] = _cache[key]
    nc = _cache[key]
    in_maps = make_in_maps(x, in_proj_w, in_proj_b, out_w, out_b)
    res = run_bass_kernel_spmd(nc, in_maps, list(range(NCORES))).results
    pieces = [res[c]["out"] for c in range(NCORES)]
    return np.concatenate(pieces, axis=0).reshape(B, T, D).astype(np.float32)


# revision 41
# speedup vs baseline: 1.1572x; 1.1572x over previous
"""Sparse multi-head self-attention (sliding window + global columns) on 8
Trainium2 NeuronCores.

Sharding: fully data-parallel over the sequence dimension. Core c produces
output rows [512c, 512c+512). Each core recomputes k/v for a 128-row halo on
each side of its slice plus the 16 global key rows (j % 256 == 0), so no
collectives are needed.

v3: all matmul operands fp16 (PE runs 1 cycle/col vs multi-pass fp32 HIGH
mode). RoPE'd activations round-trip through DRAM so the din-major transpose
is 8 bulk DMA-transposes instead of 56 PE transposes + copies. v is computed
directly in natural [key, hd] orientation (xT slices stationary). Scores for
the two heads of a pair run concurrently in disjoint PE row groups. Softmax
normalization: denominator via an appended ones-column of v, fast-approx
reciprocal on DVE, partition-broadcast on GpSimd. RoPE adds on GpSimd.
"""
import sys

sys.path.insert(0, "/opt/trn_rl_repo")

import numpy as np
import concourse.bass as bass
import concourse.mybir as mybir
from concourse.tile import TileContext

# ---------------------------------------------------------------- constants
B, T, D = 1, 4096, 1024
H, HD = 16, 64
W = 128
GSTRIDE = 256
ROPE_BASE = 10000.0
NCORES = 8
TLOC = T // NCORES            # 512 own rows per core
HALO = 128
NL = TLOC + 2 * HALO          # 768 rows incl. halo
NG = T // GSTRIDE             # 16 global keys
NR = NL + NG                  # 784 rows incl. globals
NT = NL // 128                # 6 local 128-row key tiles
NQB = TLOC // 128             # 4 query blocks per core
NDT = D // 128                # 8 din tiles
SCALE = 1.0 / np.sqrt(HD)

# per-m query-column ranges within the core's 512 own rows
QS = [0, 0, 0, 128, 256, 384]
QW = [128, 256, 384, 384, 256, 128]

F32 = mybir.dt.float32
F16 = mybir.dt.float16
FT = mybir.ActivationFunctionType

_cache = {}


# ------------------------------------------------------- walrus workaround
def _fix_multi_waits(nc):
    """This walrus build encodes at most ONE sem wait per instruction; hoist
    extra waits onto same-engine NoOps inserted just before the owner."""
    count = 0
    for fn in nc.m.functions:
        for bb in fn.blocks:
            old = bb.instructions
            if not any(
                i.sync_info is not None and len(i.sync_info.on_wait or []) > 1
                for i in old
            ):
                continue
            new = []
            for inst in old:
                si = inst.sync_info
                waits = list(si.on_wait) if si is not None and si.on_wait else []
                if len(waits) > 1:
                    for w in waits[:-1]:
                        count += 1
                        new.append(
                            mybir.InstNoOp(
                                name=f"I-waitfix-{count}",
                                engine=inst.engine,
                                bass_nofuse=True,
                                sync_info=mybir.SyncInfo(on_wait=[w], on_update=[]),
                            )
                        )
                    inst.sync_info = mybir.SyncInfo(
                        on_wait=[waits[-1]], on_update=list(si.on_update or [])
                    )
                new.append(inst)
            bb.instructions = new
    return count


def _bcast_mid(ap2d, reps):
    """[P, F] AP -> [P, reps, F] AP broadcasting along a middle free dim."""
    a = [list(x) for x in ap2d.ap]
    return bass.AP(tensor=ap2d.tensor, offset=ap2d.offset,
                   ap=[a[0], [0, reps], a[1]])


# ------------------------------------------------------------ bass program
def build_program(has_vbias, has_obias, debug=False):
    nc = bass.Bass()

    xd = nc.dram_tensor("xd", [NR, D], F16, kind="ExternalInput")
    csd = nc.dram_tensor("csd", [NR, 64], F16, kind="ExternalInput")
    identd = nc.dram_tensor("identd", [128, 128], F16, kind="ExternalInput")
    wts = nc.dram_tensor("wts", [24, 128, D], F16, kind="ExternalInput")
    wosd = nc.dram_tensor("wosd", [16, 128, 512], F16, kind="ExternalInput")
    ball = nc.dram_tensor("ball", [128, 24], F16, kind="ExternalInput")
    bvp = nc.dram_tensor("bvp", [1, D], F16, kind="ExternalInput")
    bo = nc.dram_tensor("bo", [1, D], F16, kind="ExternalInput")
    mloc = nc.dram_tensor("mloc", [128, NT * 384], F16, kind="ExternalInput")
    mgp = nc.dram_tensor("mgp", [48, TLOC], F16, kind="ExternalInput")
    sel8 = nc.dram_tensor("sel8", [8, 4 * 128], F16, kind="ExternalInput")
    out = nc.dram_tensor("out", [TLOC, D], F32, kind="ExternalOutput")

    with TileContext(nc) as tc:
        _build_body(nc, tc, xd, csd, identd, wts, wosd, ball, bvp, bo, mloc,
                    mgp, sel8, out, has_vbias, has_obias, debug)
    _fix_multi_waits(nc)
    return nc


def _build_body(nc, tc, xd, csd, identd, wts, wosd, ball, bvp, bo, mloc,
                mgp, sel8, out, has_vbias, has_obias, debug=False):
    from contextlib import ExitStack
    ctx = ExitStack()
    with ctx:
        singles = ctx.enter_context(tc.tile_pool(name="singles", bufs=1))
        ppool = ctx.enter_context(tc.tile_pool(name="ppool", bufs=4))
        npool = ctx.enter_context(tc.tile_pool(name="npool", bufs=2))
        opool = ctx.enter_context(tc.tile_pool(name="opool", bufs=2))
        ps_mm = ctx.enter_context(tc.tile_pool(name="ps_mm", bufs=2, space="PSUM"))
        ps_s = ctx.enter_context(tc.tile_pool(name="ps_s", bufs=3, space="PSUM"))
        ps_g = ctx.enter_context(tc.tile_pool(name="ps_g", bufs=1, space="PSUM"))
        ps_o = ctx.enter_context(tc.tile_pool(name="ps_o", bufs=2, space="PSUM"))

        # ---------------- RoPE (x layout: [evens 512 | odds 512]) + PE
        # transpose per tile into xT slices
        HD2 = D // 2
        ident = singles.tile([128, 128], F16)
        nc.sync.dma_start(ident[:], identd[:])
        xT = [singles.tile([128, NR], F16, tag=f"xT{k}", name=f"xT{k}")
              for k in range(NDT)]
        with tc.tile_pool(name="ropepool", bufs=2) as rp:
            ps_tr = ps_s
            for i in range(NT + 1):
                p = 128 if i < NT else NG
                r0 = i * 128
                xc = rp.tile([128, D], F16, tag="xc", name=f"xc{i}")
                nc.sync.dma_start(xc[:p], xd[r0:r0 + p, :])
                cs = rp.tile([128, 64], F16, tag="cs", name=f"cs{i}")
                nc.sync.dma_start(cs[:p], csd[r0:r0 + p, :])
                roped = rp.tile([128, D], F16, tag="roped", name=f"roped{i}")
                tmp = rp.tile([128, D], F16, tag="ropetmp", name=f"ropetmp{i}")
                xe = xc[:p, 0:HD2].rearrange("p (h d) -> p h d", h=H)
                xo = xc[:p, HD2:D].rearrange("p (h d) -> p h d", h=H)
                te = tmp[:p, 0:HD2].rearrange("p (h d) -> p h d", h=H)
                to = tmp[:p, HD2:D].rearrange("p (h d) -> p h d", h=H)
                re_ = roped[:p, 0:HD2].rearrange("p (h d) -> p h d", h=H)
                ro = roped[:p, HD2:D].rearrange("p (h d) -> p h d", h=H)
                cosb = _bcast_mid(cs[:p, 0:32], H)
                sinb = _bcast_mid(cs[:p, 32:64], H)
                nc.vector.tensor_mul(te, xe, cosb)
                nc.vector.tensor_mul(to, xo, sinb)
                nc.vector.tensor_sub(re_, te, to)
                nc.vector.tensor_mul(te, xe, sinb)
                nc.vector.tensor_mul(to, xo, cosb)
                nc.vector.tensor_add(ro, te, to)
                for k in range(NDT):
                    ptr = ps_tr.tile([128, 128], F16, tag="s",
                                     name=f"ptr{i}_{k}")
                    if p == 128:
                        nc.tensor.transpose(ptr[:], roped[:, k * 128:(k + 1) * 128],
                                            ident[:])
                        nc.scalar.copy(xT[k][:, r0:r0 + 128], ptr[:])
                    else:
                        nc.tensor.transpose(ptr[:, 0:p],
                                            roped[0:p, k * 128:(k + 1) * 128],
                                            ident[0:p, 0:p])
                        nc.scalar.copy(xT[k][:, r0:r0 + p], ptr[:, 0:p])

        # ---------------- constants / weights (issued on the ACT queue so
        # the sync queue serves the rope/transpose critical path)
        ones = singles.tile([1, 128], F16)
        nc.vector.memset(ones[:], 1.0)
        sel8_sb = singles.tile([8, 4, 128], F16)
        nc.scalar.dma_start(sel8_sb[:], sel8[:].rearrange("p (v q) -> p v q", v=4))
        ball_sb = singles.tile([128, 24], F16)
        nc.scalar.dma_start(ball_sb[:], ball[:])
        bvp_sb = singles.tile([1, D], F16)
        nc.scalar.dma_start(bvp_sb[:], bvp[:])
        bo_sb = singles.tile([1, D], F16)
        nc.scalar.dma_start(bo_sb[:], bo[:])
        mask_sb = singles.tile([128, NT, 384], F16)
        nc.scalar.dma_start(mask_sb[:], mloc[:].rearrange("p (m q) -> p m q", m=NT))
        mgp_sb = singles.tile([48, TLOC], F16)
        nc.scalar.dma_start(mgp_sb[:], mgp[:])
        wq_sb = [singles.tile([128, D], F16, tag=f"wq{i}", name=f"wq{i}")
                 for i in range(NDT)]
        wk_sb = [singles.tile([128, D], F16, tag=f"wk{i}", name=f"wk{i}")
                 for i in range(NDT)]
        wv_sb = [singles.tile([128, D], F16, tag=f"wv{i}", name=f"wv{i}")
                 for i in range(NDT)]
        for i in range(NDT):
            nc.scalar.dma_start(wv_sb[i][:], wts[16 + i])
        for i in range(NDT):
            nc.scalar.dma_start(wq_sb[i][:], wts[i])
            nc.scalar.dma_start(wk_sb[i][:], wts[8 + i])
        wos_sb = [singles.tile([128, 512], F16, tag=f"wos{i}", name=f"wos{i}")
                  for i in range(16)]
        for i in range(16):
            nc.scalar.dma_start(wos_sb[i][:], wosd[i])

        # persistent per-pair tensors
        qT = [singles.tile([128, TLOC], F16, tag=f"qT{c}", name=f"qT{c}")
              for c in range(NDT)]
        kT = [singles.tile([128, NR], F16, tag=f"kT{c}", name=f"kT{c}")
              for c in range(NDT)]
        # v natural: per key tile, [keys, hp, head, 64+ones]
        v_sb = [singles.tile([128, NDT, 2, HD + 1], F16, tag=f"v{m}",
                             name=f"v{m}") for m in range(NT)]
        vg_sb = singles.tile([48, NDT, HD + 1], F16)
        for m in range(NT):
            nc.vector.memset(v_sb[m][:, :, :, HD:HD + 1], 1.0)
        nc.vector.memset(vg_sb[:, :, HD:HD + 1], 1.0)
        oT = [singles.tile([128, TLOC], F16, tag=f"oT{k}", name=f"oT{k}")
              for k in range(NDT)]
        BATCHES = [(0, 1, 2, 3), (4, 5), (6, 7)]
        HP2BAT = {hp: (b, bat.index(hp) * 2)
                  for b, bat in enumerate(BATCHES) for hp in bat}
        denall = [singles.tile([2 * len(bat), TLOC], F32, tag=f"den{b}",
                               name=f"den{b}") for b, bat in enumerate(BATCHES)]
        rcp16 = [singles.tile([2 * len(bat), TLOC], F16, tag=f"rcp{b}",
                              name=f"rcp{b}") for b, bat in enumerate(BATCHES)]

        # ---------------- v projection, natural orientation, all pairs at
        # once: stationary xT key-block streams 512 dout columns twice
        for m in range(NT):
            for half in range(2):
                pv = ps_mm.tile([128, 512], F32, tag="mm",
                                name=f"pv{m}_{half}")
                for k in range(NDT):
                    nc.tensor.matmul(pv[:], xT[k][:, m * 128:(m + 1) * 128],
                                     wv_sb[k][:, half * 512:(half + 1) * 512],
                                     start=(k == 0), stop=(k == NDT - 1))
                nc.scalar.copy(
                    v_sb[m][:, 4 * half:4 * half + 4, :, 0:HD],
                    pv[:].rearrange("p (a b c) -> p a b c", a=4, b=2))
        for half in range(2):
            pvg = ps_mm.tile([128, 512], F32, tag="mm", name=f"pvg{half}")
            for k in range(NDT):
                nc.tensor.matmul(pvg[0:NG, :], xT[k][:, NL:NR],
                                 wv_sb[k][:, half * 512:(half + 1) * 512],
                                 start=(k == 0), stop=(k == NDT - 1))
            p3 = pvg[0:NG, :].rearrange("p (a c) -> p a c", a=8)
            nc.scalar.copy(vg_sb[0:NG, 4 * half:4 * half + 4, 0:HD],
                           p3[:, 0::2, :])
            vgst = npool.tile([NG, 4, HD], F16, tag="vgst", name=f"vgst{half}")
            nc.scalar.copy(vgst[:], p3[:, 1::2, :])
            nc.sync.dma_start(vg_sb[32:48, 4 * half:4 * half + 4, 0:HD],
                              vgst[:])

        # ---------------- main pipeline: per head-pair hp
        for hp in range(NDT):
            # --- q projection (own 512 rows)
            pq = ps_mm.tile([128, 512], F32, tag="mm", name=f"pq{hp}")
            for k in range(NDT):
                nc.tensor.matmul(pq[:], wq_sb[k][:, hp * 128:(hp + 1) * 128],
                                 xT[k][:, HALO:HALO + TLOC],
                                 start=(k == 0), stop=(k == NDT - 1))
            nc.scalar.add(qT[hp][:], pq[:], ball_sb[:, hp:hp + 1])

            # --- k projection (768 halo rows + 16 globals)
            pk0 = ps_mm.tile([128, 512], F32, tag="mm", name=f"pk0{hp}")
            for k in range(NDT):
                nc.tensor.matmul(pk0[:], wk_sb[k][:, hp * 128:(hp + 1) * 128],
                                 xT[k][:, 0:512],
                                 start=(k == 0), stop=(k == NDT - 1))
            nc.scalar.add(kT[hp][:, 0:512], pk0[:], ball_sb[:, 8 + hp:9 + hp])
            pk1 = ps_mm.tile([128, 512], F32, tag="mm", name=f"pk1{hp}")
            for k in range(NDT):
                nc.tensor.matmul(pk1[:, 0:NR - 512],
                                 wk_sb[k][:, hp * 128:(hp + 1) * 128],
                                 xT[k][:, 512:NR],
                                 start=(k == 0), stop=(k == NDT - 1))
            nc.scalar.add(kT[hp][:, 512:NR], pk1[:, 0:NR - 512],
                          ball_sb[:, 8 + hp:9 + hp])

            # --- attention: scores for both heads of the pair run in
            # disjoint PE row groups; pv interleaved per key tile
            po_a = ps_o.tile([HD + 1, TLOC], F32, tag="o", name=f"poa{hp}")
            po_b = ps_o.tile([HD + 1, TLOC], F32, tag="o", name=f"pob{hp}")
            for m in range(NT):
                w, qs = QW[m], QS[m]
                psc_a = ps_s.tile([128, 384], F32, tag="s", name=f"psa{hp}_{m}")
                psc_b = ps_s.tile([128, 384], F32, tag="s", name=f"psb{hp}_{m}")
                nc.tensor.matmul(psc_a[:, 0:w],
                                 kT[hp][0:64, m * 128:(m + 1) * 128],
                                 qT[hp][0:64, qs:qs + w], start=True, stop=True)
                nc.tensor.matmul(psc_b[:, 0:w],
                                 kT[hp][64:128, m * 128:(m + 1) * 128],
                                 qT[hp][64:128, qs:qs + w], start=True, stop=True)
                nc.vector.tensor_add(psc_a[:, 0:w], psc_a[:, 0:w],
                                     mask_sb[:, m, 0:w])
                nc.vector.tensor_add(psc_b[:, 0:w], psc_b[:, 0:w],
                                     mask_sb[:, m, 0:w])
                ea = ppool.tile([128, 384], F16, tag="pe", name=f"pea{hp}_{m}")
                eb = ppool.tile([128, 384], F16, tag="pe", name=f"peb{hp}_{m}")
                nc.scalar.activation(ea[:, 0:w], psc_a[:, 0:w], FT.Exp)
                nc.scalar.activation(eb[:, 0:w], psc_b[:, 0:w], FT.Exp)
                nc.tensor.matmul(po_a[:, qs:qs + w], v_sb[m][:, hp, 0, :],
                                 ea[:, 0:w], start=(m == 0), stop=False)
                nc.tensor.matmul(po_b[:, qs:qs + w], v_sb[m][:, hp, 1, :],
                                 eb[:, 0:w], start=(m == 0), stop=False)

            psg = ps_g.tile([48, TLOC], F32, tag="g", name=f"psg{hp}")
            nc.tensor.matmul(psg[0:NG, :], kT[hp][0:64, NL:NR],
                             qT[hp][0:64, :], start=True, stop=True)
            nc.tensor.matmul(psg[32:48, :], kT[hp][64:128, NL:NR],
                             qT[hp][64:128, :], start=True, stop=True,
                             skip_group_check=True)
            nc.vector.tensor_add(psg[:], psg[:], mgp_sb[:])
            peg = ppool.tile([48, TLOC], F16, tag="peg", name=f"peg{hp}")
            nc.scalar.activation(peg[:], psg[:], FT.Exp)

            # --- global pv + denominator extraction + unnormalized copy-out
            bat, slot = HP2BAT[hp]
            for j in range(2):
                po = po_a if j == 0 else po_b
                gb = 32 * j
                nc.tensor.matmul(po[:], vg_sb[gb:gb + NG, hp, :],
                                 peg[gb:gb + NG, :], start=False,
                                 stop=(not has_vbias))
                dn = npool.tile([1, TLOC], F32, tag="dn", name=f"dn{hp}_{j}")
                nc.scalar.copy(dn[:], po[HD:HD + 1, :])
                nc.sync.dma_start(denall[bat][slot + j:slot + j + 1, :], dn[:])
                if has_vbias:
                    denh = npool.tile([1, TLOC], F16, tag="denh",
                                      name=f"denh{hp}_{j}")
                    nc.scalar.copy(denh[:], po[HD:HD + 1, :])
                    nc.tensor.matmul(po[0:HD, :],
                                     bvp_sb[:, hp * 128 + 64 * j:
                                            hp * 128 + 64 * j + HD],
                                     denh[:], start=False, stop=True,
                                     skip_group_check=True)
                if j == 0:
                    nc.vector.tensor_copy(oT[hp][0:64, :], po[0:HD, :])
                else:
                    ost = npool.tile([64, TLOC], F16, tag="ost",
                                     name=f"ost{hp}")
                    nc.vector.tensor_copy(ost[:], po[0:HD, :])
                    nc.sync.dma_start(oT[hp][64:128, :], ost[:])

            # batched reciprocal at batch boundaries, then in-place normalize
            if hp == BATCHES[bat][-1]:
                nc.vector.reciprocal(denall[bat][:], denall[bat][:])
                nc.vector.tensor_copy(rcp16[bat][:], denall[bat][:])
                for vi, php in enumerate(BATCHES[bat]):
                    pbb = ps_g.tile([128, TLOC], F32, tag="g",
                                    name=f"pb{php}")
                    nc.tensor.matmul(pbb[:], sel8_sb[0:2 * len(BATCHES[bat]),
                                                     vi, :],
                                     rcp16[bat][:],
                                     start=True, stop=True)
                    bc = npool.tile([128, TLOC], F16, tag="bc",
                                    name=f"bc{php}")
                    nc.vector.tensor_copy(bc[:], pbb[:])
                    nc.vector.tensor_mul(oT[php][0:64, :],
                                         oT[php][0:64, :], bc[0:64, :])
                    nc.vector.tensor_mul(oT[php][64:128, :],
                                         oT[php][64:128, :], bc[64:128, :])

        if debug:
            dxT = nc.dram_tensor("dxT", [128, NDT, NR], F16,
                                 kind="ExternalOutput")
            dq = nc.dram_tensor("dq", [128, NDT, TLOC], F16,
                                kind="ExternalOutput")
            dk = nc.dram_tensor("dk", [128, NDT, NR], F16,
                                kind="ExternalOutput")
            dv = nc.dram_tensor("dv", [128, NT, NDT, 2, HD + 1], F16,
                                kind="ExternalOutput")
            dvg = nc.dram_tensor("dvg", [48, NDT, HD + 1], F16,
                                 kind="ExternalOutput")
            doT = nc.dram_tensor("doT", [128, NDT, TLOC], F16,
                                 kind="ExternalOutput")
            for b, bat_ in enumerate(BATCHES):
                drc = nc.dram_tensor(f"drc{b}", [2 * len(bat_), TLOC], F32,
                                     kind="ExternalOutput")
                nc.sync.dma_start(drc[:], denall[b][:])
            for k in range(NDT):
                nc.sync.dma_start(dxT[:, k, :], xT[k][:])
                nc.sync.dma_start(dq[:, k, :], qT[k][:])
                nc.sync.dma_start(dk[:, k, :], kT[k][:])
                nc.sync.dma_start(doT[:, k, :], oT[k][:])
            for m in range(NT):
                nc.sync.dma_start(dv[:, m], v_sb[m][:])
            nc.sync.dma_start(dvg[:], vg_sb[:])

        # ---------------- output projection
        for qb in range(NQB):
            for chp in range(2):
                pout = ps_mm.tile([128, 512], F32, tag="mm",
                                  name=f"pout{qb}_{chp}")
                for k in range(NDT):
                    nc.tensor.matmul(pout[:], oT[k][:, qb * 128:(qb + 1) * 128],
                                     wos_sb[chp * 8 + k][:],
                                     start=(k == 0),
                                     stop=(k == NDT - 1 and not has_obias))
                if has_obias:
                    nc.tensor.matmul(pout[:], ones[:, 0:128],
                                     bo_sb[:, chp * 512:(chp + 1) * 512],
                                     start=False, stop=True)
                so = opool.tile([128, 512], F32, tag="outsb",
                                name=f"so{qb}_{chp}")
                nc.scalar.copy(so[:], pout[:])
                nc.sync.dma_start(
                    out[qb * 128:(qb + 1) * 128, chp * 512:(chp + 1) * 512],
                    so[:])


# ------------------------------------------------------------ host helpers
def _perm():
    # global evens-then-odds layout: col h*32+i = x[h*64+2i],
    # col 512+h*32+i = x[h*64+2i+1]
    p = np.arange(D).reshape(H, 32, 2)
    return np.concatenate([p[:, :, 0].reshape(-1), p[:, :, 1].reshape(-1)])


def _cos_sin(trows):
    """Tables matching the reference's quirky emb[..., ::2] indexing."""
    inv_freq = (1.0 / (ROPE_BASE ** (np.arange(0, HD, 2, dtype=np.float32) / HD))
                ).astype(np.float32)
    pos = trows.astype(np.float32)
    freqs = pos[:, None] * inv_freq[None, :]
    emb = np.concatenate([freqs, freqs], axis=-1)[:, ::2]      # (n, 32)
    return np.cos(emb).astype(np.float32), np.sin(emb).astype(np.float32)


def _allowed(i, j):
    ok = (np.abs(i - j) <= W) | (j % GSTRIDE == 0) | (j == 0)
    return ok & (j >= 0) & (j < T)


NEG = np.float16(-30000.0)


def make_in_maps(x, in_proj_w, in_proj_b, out_w, out_b):
    perm = _perm()
    x2 = np.asarray(x, np.float32).reshape(T, D)[:, perm]
    wp = np.asarray(in_proj_w, np.float32)[:, perm]
    wt_full = np.ascontiguousarray(wp.T).astype(np.float32)     # (D, 3D)
    wt_full[:, 0:D] *= SCALE
    # wts[8*s + i] = wt_full[128i:128(i+1), s*D:(s+1)*D]  (din-chunk major)
    wts = np.ascontiguousarray(np.concatenate(
        [wt_full[:, s:s + D].reshape(NDT, 128, D) for s in (0, D, 2 * D)],
        axis=0)).astype(np.float16)
    b = np.asarray(in_proj_b, np.float32).copy()
    b[0:D] *= SCALE
    ball = np.ascontiguousarray(b.reshape(24, 128).T).astype(np.float16)
    bvp = np.ascontiguousarray(b[2 * D:][None, :]).astype(np.float16)
    wo_full = np.ascontiguousarray(np.asarray(out_w, np.float32).T)  # (din,dout)
    # wos[8*chp + k] = wo_full[128k:128(k+1), 512chp:512(chp+1)]
    wos = np.ascontiguousarray(
        wo_full.reshape(NDT, 128, 2, 512).transpose(2, 0, 1, 3)
        .reshape(16, 128, 512)).astype(np.float16)
    bo = np.ascontiguousarray(np.asarray(out_b, np.float32)[None, :]
                              ).astype(np.float16)

    tg = np.arange(NG) * GSTRIDE
    cg, sg = _cos_sin(tg)
    csg = np.concatenate([cg, sg], axis=1)

    sel8 = np.zeros((8, 4, 128), np.float16)
    for v in range(4):
        sel8[2 * v, v, 0:64] = 1
        sel8[2 * v + 1, v, 64:128] = 1
    sel8 = np.ascontiguousarray(sel8.reshape(8, 512))

    ident = np.eye(128, dtype=np.float16)

    in_maps = []
    for c in range(NCORES):
        t0 = c * TLOC - HALO
        rows = np.arange(t0, t0 + NL)
        valid = (rows >= 0) & (rows < T)
        xdc = np.zeros((NR, D), np.float32)
        xdc[:NL][valid] = x2[rows[valid]]
        xdc[NL:] = x2[tg]
        cl, sl = _cos_sin(np.clip(rows, 0, T - 1))
        csdc = np.concatenate(
            [np.concatenate([cl, sl], axis=1), csg], axis=0)

        ml = np.full((NT, 128, 384), NEG, np.float16)
        for m in range(NT):
            jj = (t0 + m * 128) + np.arange(128)
            ii = c * TLOC + QS[m] + np.arange(QW[m])
            ml[m, :, 0:QW[m]] = np.where(
                _allowed(ii[None, :], jj[:, None]), np.float16(0), NEG)
        mloc = np.ascontiguousarray(ml.transpose(1, 0, 2).reshape(128, NT * 384))
        iq = c * TLOC + np.arange(TLOC)
        qb = iq // 128
        jg = tg[:, None]
        covered = (jg >= 128 * (qb[None, :] - 1)) & (jg < 128 * (qb[None, :] + 2))
        mg1 = np.where(covered, NEG, np.float16(0)).astype(np.float16)
        mgp = np.full((48, TLOC), NEG, np.float16)
        mgp[0:16] = mg1
        mgp[32:48] = mg1

        in_maps.append({
            "xd": xdc.astype(np.float16), "csd": csdc.astype(np.float16),
            "identd": ident, "wts": wts, "wosd": wos,
            "ball": ball, "bvp": bvp, "bo": bo, "mloc": mloc, "mgp": mgp,
            "sel8": sel8,
        })
    return in_maps


def kernel(x, in_proj_w, in_proj_b, out_w, out_b):
    from concourse.bass_utils import run_bass_kernel_spmd

    has_vbias = bool(np.any(np.asarray(in_proj_b)[2 * D:] != 0))
    has_obias = bool(np.any(np.asarray(out_b) != 0))
    key = ("nc", has_vbias, has_obias)
    if key not in _cache:
        _cache[key] = build_program(has_vbias, has_obias)
        _cache["nc"] = _cache[key]
    nc = _cache[key]
    in_maps = make_in_maps(x, in_proj_w, in_proj_b, out_w, out_b)
    res = run_bass_kernel_spmd(nc, in_maps, list(range(NCORES))).results
    pieces = [res[c]["out"] for c in range(NCORES)]
    return np.concatenate(pieces, axis=0).reshape(B, T, D).astype(np.float32)


# revision 64
# speedup vs baseline: 1.2608x; 1.0895x over previous
"""Sparse multi-head self-attention (sliding window + global columns) on 8
Trainium2 NeuronCores.

Sharding: fully data-parallel over the sequence dimension. Core c produces
output rows [512c, 512c+512). Each core recomputes k/v for a 128-row halo on
each side of its slice plus the 16 global key rows (j % 256 == 0), so no
collectives are needed.

v3: all matmul operands fp16 (PE runs 1 cycle/col vs multi-pass fp32 HIGH
mode). RoPE'd activations round-trip through DRAM so the din-major transpose
is 8 bulk DMA-transposes instead of 56 PE transposes + copies. v is computed
directly in natural [key, hd] orientation (xT slices stationary). Scores for
the two heads of a pair run concurrently in disjoint PE row groups. Softmax
normalization: denominator via an appended ones-column of v, fast-approx
reciprocal on DVE, partition-broadcast on GpSimd. RoPE adds on GpSimd.
"""
import sys

sys.path.insert(0, "/opt/trn_rl_repo")

import numpy as np
import concourse.bass as bass
import concourse.mybir as mybir
from concourse.tile import TileContext

# ---------------------------------------------------------------- constants
B, T, D = 1, 4096, 1024
H, HD = 16, 64
W = 128
GSTRIDE = 256
ROPE_BASE = 10000.0
NCORES = 8
TLOC = T // NCORES            # 512 own rows per core
HALO = 128
NL = TLOC + 2 * HALO          # 768 rows incl. halo
NG = T // GSTRIDE             # 16 global keys
NR = NL + NG                  # 784 rows incl. globals
NT = NL // 128                # 6 local 128-row key tiles
NQB = TLOC // 128             # 4 query blocks per core
NDT = D // 128                # 8 din tiles
SCALE = 1.0 / np.sqrt(HD)

# per-m query-column ranges within the core's 512 own rows
QS = [0, 0, 0, 128, 256, 384]
QW = [128, 256, 384, 384, 256, 128]

F32 = mybir.dt.float32
F16 = mybir.dt.float16
FT = mybir.ActivationFunctionType

_cache = {}


# ------------------------------------------------------- walrus workaround
def _fix_multi_waits(nc):
    """This walrus build encodes at most ONE sem wait per instruction; hoist
    extra waits onto same-engine NoOps inserted just before the owner."""
    count = 0
    for fn in nc.m.functions:
        for bb in fn.blocks:
            old = bb.instructions
            if not any(
                i.sync_info is not None and len(i.sync_info.on_wait or []) > 1
                for i in old
            ):
                continue
            new = []
            for inst in old:
                si = inst.sync_info
                waits = list(si.on_wait) if si is not None and si.on_wait else []
                if len(waits) > 1:
                    for w in waits[:-1]:
                        count += 1
                        new.append(
                            mybir.InstNoOp(
                                name=f"I-waitfix-{count}",
                                engine=inst.engine,
                                bass_nofuse=True,
                                sync_info=mybir.SyncInfo(on_wait=[w], on_update=[]),
                            )
                        )
                    inst.sync_info = mybir.SyncInfo(
                        on_wait=[waits[-1]], on_update=list(si.on_update or [])
                    )
                new.append(inst)
            bb.instructions = new
    return count


def _bcast_mid(ap2d, reps):
    """[P, F] AP -> [P, reps, F] AP broadcasting along a middle free dim."""
    a = [list(x) for x in ap2d.ap]
    return bass.AP(tensor=ap2d.tensor, offset=ap2d.offset,
                   ap=[a[0], [0, reps], a[1]])


# ------------------------------------------------------------ bass program
def build_program(has_vbias, has_obias, debug=False):
    nc = bass.Bass()

    xcs = nc.dram_tensor("xcs", [NR, D + 64], F16, kind="ExternalInput")
    identd = nc.dram_tensor("identd", [128, 128], F16, kind="ExternalInput")
    wall = nc.dram_tensor("wall", [128, 32 * D], F16, kind="ExternalInput")
    ball = nc.dram_tensor("ball", [128, 24], F16, kind="ExternalInput")
    bvp = nc.dram_tensor("bvp", [1, D], F16, kind="ExternalInput")
    bo = nc.dram_tensor("bo", [1, D], F16, kind="ExternalInput")
    mloc = nc.dram_tensor("mloc", [128, NT * 384], F16, kind="ExternalInput")
    mgp = nc.dram_tensor("mgp", [48, TLOC], F16, kind="ExternalInput")
    sel8 = nc.dram_tensor("sel8", [8, 4 * 128], F16, kind="ExternalInput")
    out = nc.dram_tensor("out", [TLOC, D], F32, kind="ExternalOutput")

    with TileContext(nc) as tc:
        _build_body(nc, tc, xcs, identd, wall, ball, bvp, bo, mloc,
                    mgp, sel8, out, has_vbias, has_obias, debug)
    _fix_multi_waits(nc)
    return nc


def _build_body(nc, tc, xcs, identd, wall, ball, bvp, bo, mloc,
                mgp, sel8, out, has_vbias, has_obias, debug=False):
    from contextlib import ExitStack
    ctx = ExitStack()
    with ctx:
        singles = ctx.enter_context(tc.tile_pool(name="singles", bufs=1))
        ppool = ctx.enter_context(tc.tile_pool(name="ppool", bufs=4))
        npool = ctx.enter_context(tc.tile_pool(name="npool", bufs=2))
        opool = ctx.enter_context(tc.tile_pool(name="opool", bufs=2))
        ps_mm = ctx.enter_context(tc.tile_pool(name="ps_mm", bufs=2, space="PSUM"))
        ps_s = ctx.enter_context(tc.tile_pool(name="ps_s", bufs=3, space="PSUM"))
        ps_g = ctx.enter_context(tc.tile_pool(name="ps_g", bufs=1, space="PSUM"))
        ps_o = ctx.enter_context(tc.tile_pool(name="ps_o", bufs=2, space="PSUM"))

        # ---------------- RoPE (x layout: [evens 512 | odds 512]) + PE
        # transposes per tile into xT slices
        HD2 = D // 2
        ident = singles.tile([128, 128], F16)
        nc.sync.dma_start(ident[:], identd[:])
        # HAM warm-up: ~40 dependency-free matmuls keep the PE busy (and its
        # clock gate open) while the DVE-bound RoPE phase runs
        warm = singles.tile([128, 512], F16)
        nc.vector.memset(warm[:], 0.125)
        for wi in range(40):
            pw = ps_g.tile([128, TLOC], F32, tag="g", name=f"warm{wi}")
            nc.tensor.matmul(pw[:], warm[:, 0:128], warm[:],
                             start=True, stop=True)
        xT = [singles.tile([128, NR], F16, tag=f"xT{k}", name=f"xT{k}")
              for k in range(NDT)]
        with tc.tile_pool(name="ropepool", bufs=2) as rp:
            for i in range(NT + 1):
                p = 128 if i < NT else NG
                r0 = i * 128
                xc = rp.tile([128, D + 64], F16, tag="xc", name=f"xc{i}")
                nc.sync.dma_start(xc[:p], xcs[r0:r0 + p, :])
                roped = rp.tile([128, D], F16, tag="roped", name=f"roped{i}")
                tmp = rp.tile([128, D], F16, tag="ropetmp", name=f"ropetmp{i}")
                xe = xc[:p, 0:HD2].rearrange("p (h d) -> p h d", h=H)
                xo = xc[:p, HD2:D].rearrange("p (h d) -> p h d", h=H)
                te = tmp[:p, 0:HD2].rearrange("p (h d) -> p h d", h=H)
                to = tmp[:p, HD2:D].rearrange("p (h d) -> p h d", h=H)
                re_ = roped[:p, 0:HD2].rearrange("p (h d) -> p h d", h=H)
                ro = roped[:p, HD2:D].rearrange("p (h d) -> p h d", h=H)
                cosb = _bcast_mid(xc[:p, D:D + 32], H)
                sinb = _bcast_mid(xc[:p, D + 32:D + 64], H)
                nc.vector.tensor_mul(te, xe, cosb)
                nc.vector.tensor_mul(to, xo, sinb)
                nc.vector.tensor_sub(re_, te, to)
                nc.vector.tensor_mul(te, xe, sinb)
                nc.vector.tensor_mul(to, xo, cosb)
                nc.vector.tensor_add(ro, te, to)
                for k in range(NDT):
                    ptr = ps_s.tile([128, 128], F16, tag="s",
                                    name=f"ptr{i}_{k}")
                    if p == 128:
                        nc.tensor.transpose(
                            ptr[:], roped[:, k * 128:(k + 1) * 128], ident[:])
                        nc.scalar.copy(xT[k][:, r0:r0 + 128], ptr[:])
                    else:
                        nc.tensor.transpose(
                            ptr[:, 0:p], roped[0:p, k * 128:(k + 1) * 128],
                            ident[0:p, 0:p])
                        nc.scalar.copy(xT[k][:, r0:r0 + p], ptr[:, 0:p])

        # ---------------- constants / weights (one bulk DMA on the ACT
        # queue; the sync queue serves the rope critical path)
        ones = singles.tile([1, 128], F16)
        nc.vector.memset(ones[:], 1.0)
        sel8_sb = singles.tile([8, 4, 128], F16)
        nc.scalar.dma_start(sel8_sb[:], sel8[:].rearrange("p (v q) -> p v q", v=4))
        ball_sb = singles.tile([128, 24], F16)
        nc.scalar.dma_start(ball_sb[:], ball[:])
        bvp_sb = singles.tile([1, D], F16)
        nc.scalar.dma_start(bvp_sb[:], bvp[:])
        bo_sb = singles.tile([1, D], F16)
        nc.scalar.dma_start(bo_sb[:], bo[:])
        mask_sb = singles.tile([128, NT, 384], F16)
        nc.scalar.dma_start(mask_sb[:], mloc[:].rearrange("p (m q) -> p m q", m=NT))
        mgp_sb = singles.tile([48, TLOC], F16)
        nc.scalar.dma_start(mgp_sb[:], mgp[:])
        # wall layout: [128, 32, 1024]: 0-7 wq, 8-15 wk, 16-23 wv din-chunks;
        # 24-31: out-proj weights, chunk 24+k = [chp0 512 | chp1 512].
        # Four separate tiles so consumers see partial availability.
        w4 = wall[:].rearrange("p (s i c) -> p s i c", s=4, i=8)
        wsec = [None] * 4
        for s in (2, 0, 1, 3):          # wv first (v-proj runs first)
            nm = ("wq", "wk", "wv", "wo")[s]
            t = singles.tile([128, 8, D], F16, tag=nm, name=nm)
            nc.scalar.dma_start(t[:], w4[:, s])
            wsec[s] = t
        wq_sb = [wsec[0][:, i, :] for i in range(NDT)]
        wk_sb = [wsec[1][:, i, :] for i in range(NDT)]
        wv_sb = [wsec[2][:, i, :] for i in range(NDT)]
        wos_sb = [wsec[3][:, i % 8, (i // 8) * 512:(i // 8) * 512 + 512]
                  for i in range(16)]

        # persistent per-pair tensors
        qT = [singles.tile([128, TLOC], F16, tag=f"qT{c}", name=f"qT{c}")
              for c in range(NDT)]
        kT = [singles.tile([128, NR], F16, tag=f"kT{c}", name=f"kT{c}")
              for c in range(NDT)]
        # v natural: per key tile, [keys, hp, head, 64+ones]
        v_sb = [singles.tile([128, NDT, 2, HD + 1], F16, tag=f"v{m}",
                             name=f"v{m}") for m in range(NT)]
        vg_sb = singles.tile([48, NDT, HD + 1], F16)
        for m in range(NT):
            nc.vector.memset(v_sb[m][:, :, :, HD:HD + 1], 1.0)
        nc.vector.memset(vg_sb[:, :, HD:HD + 1], 1.0)
        oT = [singles.tile([128, TLOC], F16, tag=f"oT{k}", name=f"oT{k}")
              for k in range(NDT)]
        BATCHES = [(0, 1, 2, 3), (4, 5), (6, 7)]
        HP2BAT = {hp: (b, bat.index(hp) * 2)
                  for b, bat in enumerate(BATCHES) for hp in bat}
        denall = [singles.tile([2 * len(bat), TLOC], F32, tag=f"den{b}",
                               name=f"den{b}") for b, bat in enumerate(BATCHES)]
        rcp16 = [singles.tile([2 * len(bat), TLOC], F16, tag=f"rcp{b}",
                              name=f"rcp{b}") for b, bat in enumerate(BATCHES)]

        # ---------------- v projection, natural orientation, all pairs at
        # once: stationary xT key-block streams 512 dout columns twice
        for m in range(NT):
            for half in range(2):
                pv = ps_mm.tile([128, 512], F32, tag="mm",
                                name=f"pv{m}_{half}")
                for k in range(NDT):
                    nc.tensor.matmul(pv[:], xT[k][:, m * 128:(m + 1) * 128],
                                     wv_sb[k][:, half * 512:(half + 1) * 512],
                                     start=(k == 0), stop=(k == NDT - 1))
                nc.scalar.copy(
                    v_sb[m][:, 4 * half:4 * half + 4, :, 0:HD],
                    pv[:].rearrange("p (a b c) -> p a b c", a=4, b=2))
        for half in range(2):
            pvg = ps_mm.tile([128, 512], F32, tag="mm", name=f"pvg{half}")
            for k in range(NDT):
                nc.tensor.matmul(pvg[0:NG, :], xT[k][:, NL:NR],
                                 wv_sb[k][:, half * 512:(half + 1) * 512],
                                 start=(k == 0), stop=(k == NDT - 1))
            p3 = pvg[0:NG, :].rearrange("p (a c) -> p a c", a=8)
            nc.scalar.copy(vg_sb[0:NG, 4 * half:4 * half + 4, 0:HD],
                           p3[:, 0::2, :])
            vgst = npool.tile([NG, 4, HD], F16, tag="vgst", name=f"vgst{half}")
            nc.scalar.copy(vgst[:], p3[:, 1::2, :])
            nc.sync.dma_start(vg_sb[32:48, 4 * half:4 * half + 4, 0:HD],
                              vgst[:])

        # ---------------- main pipeline: per head-pair hp
        for hp in range(NDT):
            # --- q projection (own 512 rows)
            pq = ps_mm.tile([128, 512], F32, tag="mm", name=f"pq{hp}")
            for k in range(NDT):
                nc.tensor.matmul(pq[:], wq_sb[k][:, hp * 128:(hp + 1) * 128],
                                 xT[k][:, HALO:HALO + TLOC],
                                 start=(k == 0), stop=(k == NDT - 1))
            nc.scalar.add(qT[hp][:], pq[:], ball_sb[:, hp:hp + 1])

            # --- k projection (768 halo rows + 16 globals)
            pk0 = ps_mm.tile([128, 512], F32, tag="mm", name=f"pk0{hp}")
            for k in range(NDT):
                nc.tensor.matmul(pk0[:], wk_sb[k][:, hp * 128:(hp + 1) * 128],
                                 xT[k][:, 0:512],
                                 start=(k == 0), stop=(k == NDT - 1))
            nc.scalar.add(kT[hp][:, 0:512], pk0[:], ball_sb[:, 8 + hp:9 + hp])
            pk1 = ps_mm.tile([128, 512], F32, tag="mm", name=f"pk1{hp}")
            for k in range(NDT):
                nc.tensor.matmul(pk1[:, 0:NR - 512],
                                 wk_sb[k][:, hp * 128:(hp + 1) * 128],
                                 xT[k][:, 512:NR],
                                 start=(k == 0), stop=(k == NDT - 1))
            nc.scalar.add(kT[hp][:, 512:NR], pk1[:, 0:NR - 512],
                          ball_sb[:, 8 + hp:9 + hp])

            # --- attention: scores for both heads of the pair run in
            # disjoint PE row groups; pv interleaved per key tile
            po_a = ps_o.tile([HD + 1, TLOC], F32, tag="o", name=f"poa{hp}")
            po_b = ps_o.tile([HD + 1, TLOC], F32, tag="o", name=f"pob{hp}")
            for m in range(NT):
                w, qs = QW[m], QS[m]
                psc_a = ps_s.tile([128, 384], F32, tag="s", name=f"psa{hp}_{m}")
                psc_b = ps_s.tile([128, 384], F32, tag="s", name=f"psb{hp}_{m}")
                nc.tensor.matmul(psc_a[:, 0:w],
                                 kT[hp][0:64, m * 128:(m + 1) * 128],
                                 qT[hp][0:64, qs:qs + w], start=True, stop=True)
                nc.tensor.matmul(psc_b[:, 0:w],
                                 kT[hp][64:128, m * 128:(m + 1) * 128],
                                 qT[hp][64:128, qs:qs + w], start=True, stop=True)
                nc.vector.tensor_add(psc_a[:, 0:w], psc_a[:, 0:w],
                                     mask_sb[:, m, 0:w])
                nc.vector.tensor_add(psc_b[:, 0:w], psc_b[:, 0:w],
                                     mask_sb[:, m, 0:w])
                ea = ppool.tile([128, 384], F16, tag="pe", name=f"pea{hp}_{m}")
                eb = ppool.tile([128, 384], F16, tag="pe", name=f"peb{hp}_{m}")
                nc.scalar.activation(ea[:, 0:w], psc_a[:, 0:w], FT.Exp)
                nc.scalar.activation(eb[:, 0:w], psc_b[:, 0:w], FT.Exp)
                nc.tensor.matmul(po_a[:, qs:qs + w], v_sb[m][:, hp, 0, :],
                                 ea[:, 0:w], start=(m == 0), stop=False)
                nc.tensor.matmul(po_b[:, qs:qs + w], v_sb[m][:, hp, 1, :],
                                 eb[:, 0:w], start=(m == 0), stop=False)

            psg = ps_g.tile([48, TLOC], F32, tag="g", name=f"psg{hp}")
            nc.tensor.matmul(psg[0:NG, :], kT[hp][0:64, NL:NR],
                             qT[hp][0:64, :], start=True, stop=True)
            nc.tensor.matmul(psg[32:48, :], kT[hp][64:128, NL:NR],
                             qT[hp][64:128, :], start=True, stop=True,
                             skip_group_check=True)
            nc.vector.tensor_add(psg[:], psg[:], mgp_sb[:])
            peg = ppool.tile([48, TLOC], F16, tag="peg", name=f"peg{hp}")
            nc.scalar.activation(peg[:], psg[:], FT.Exp)

            # --- global pv + denominator extraction + unnormalized copy-out
            bat, slot = HP2BAT[hp]
            for j in range(2):
                po = po_a if j == 0 else po_b
                gb = 32 * j
                nc.tensor.matmul(po[:], vg_sb[gb:gb + NG, hp, :],
                                 peg[gb:gb + NG, :], start=False,
                                 stop=(not has_vbias))
                dn = npool.tile([1, TLOC], F32, tag="dn", name=f"dn{hp}_{j}")
                nc.scalar.copy(dn[:], po[HD:HD + 1, :])
                nc.sync.dma_start(denall[bat][slot + j:slot + j + 1, :], dn[:])
                if has_vbias:
                    denh = npool.tile([1, TLOC], F16, tag="denh",
                                      name=f"denh{hp}_{j}")
                    nc.scalar.copy(denh[:], po[HD:HD + 1, :])
                    nc.tensor.matmul(po[0:HD, :],
                                     bvp_sb[:, hp * 128 + 64 * j:
                                            hp * 128 + 64 * j + HD],
                                     denh[:], start=False, stop=True,
                                     skip_group_check=True)
                if j == 0:
                    nc.vector.tensor_copy(oT[hp][0:64, :], po[0:HD, :])
                else:
                    ost = npool.tile([64, TLOC], F16, tag="ost",
                                     name=f"ost{hp}")
                    nc.vector.tensor_copy(ost[:], po[0:HD, :])
                    nc.sync.dma_start(oT[hp][64:128, :], ost[:])

            # batched reciprocal at batch boundaries, then in-place normalize
            if hp == BATCHES[bat][-1]:
                nc.vector.reciprocal(denall[bat][:], denall[bat][:])
                nc.vector.tensor_copy(rcp16[bat][:], denall[bat][:])
                for vi, php in enumerate(BATCHES[bat]):
                    pbb = ps_g.tile([128, TLOC], F32, tag="g",
                                    name=f"pb{php}")
                    nc.tensor.matmul(pbb[:], sel8_sb[0:2 * len(BATCHES[bat]),
                                                     vi, :],
                                     rcp16[bat][:],
                                     start=True, stop=True)
                    bc = npool.tile([128, TLOC], F16, tag="bc",
                                    name=f"bc{php}")
                    nc.vector.tensor_copy(bc[:], pbb[:])
                    nc.vector.tensor_mul(oT[php][0:64, :],
                                         oT[php][0:64, :], bc[0:64, :])
                    nc.vector.tensor_mul(oT[php][64:128, :],
                                         oT[php][64:128, :], bc[64:128, :])

        if debug:
            dxT = nc.dram_tensor("dxT", [128, NDT, NR], F16,
                                 kind="ExternalOutput")
            dq = nc.dram_tensor("dq", [128, NDT, TLOC], F16,
                                kind="ExternalOutput")
            dk = nc.dram_tensor("dk", [128, NDT, NR], F16,
                                kind="ExternalOutput")
            dv = nc.dram_tensor("dv", [128, NT, NDT, 2, HD + 1], F16,
                                kind="ExternalOutput")
            dvg = nc.dram_tensor("dvg", [48, NDT, HD + 1], F16,
                                 kind="ExternalOutput")
            doT = nc.dram_tensor("doT", [128, NDT, TLOC], F16,
                                 kind="ExternalOutput")
            for b, bat_ in enumerate(BATCHES):
                drc = nc.dram_tensor(f"drc{b}", [2 * len(bat_), TLOC], F32,
                                     kind="ExternalOutput")
                nc.sync.dma_start(drc[:], denall[b][:])
            for k in range(NDT):
                nc.sync.dma_start(dxT[:, k, :], xT[k][:])
                nc.sync.dma_start(dq[:, k, :], qT[k][:])
                nc.sync.dma_start(dk[:, k, :], kT[k][:])
                nc.sync.dma_start(doT[:, k, :], oT[k][:])
            for m in range(NT):
                nc.sync.dma_start(dv[:, m], v_sb[m][:])
            nc.sync.dma_start(dvg[:], vg_sb[:])

        # ---------------- output projection
        for qb in range(NQB):
            for chp in range(2):
                pout = ps_mm.tile([128, 512], F32, tag="mm",
                                  name=f"pout{qb}_{chp}")
                for k in range(NDT):
                    nc.tensor.matmul(pout[:], oT[k][:, qb * 128:(qb + 1) * 128],
                                     wos_sb[chp * 8 + k][:],
                                     start=(k == 0),
                                     stop=(k == NDT - 1 and not has_obias))
                if has_obias:
                    nc.tensor.matmul(pout[:], ones[:, 0:128],
                                     bo_sb[:, chp * 512:(chp + 1) * 512],
                                     start=False, stop=True)
                so = opool.tile([128, 512], F32, tag="outsb",
                                name=f"so{qb}_{chp}")
                nc.scalar.copy(so[:], pout[:])
                nc.sync.dma_start(
                    out[qb * 128:(qb + 1) * 128, chp * 512:(chp + 1) * 512],
                    so[:])


# ------------------------------------------------------------ host helpers
def _perm():
    # global evens-then-odds layout: col h*32+i = x[h*64+2i],
    # col 512+h*32+i = x[h*64+2i+1]
    p = np.arange(D).reshape(H, 32, 2)
    return np.concatenate([p[:, :, 0].reshape(-1), p[:, :, 1].reshape(-1)])


def _cos_sin(trows):
    """Tables matching the reference's quirky emb[..., ::2] indexing."""
    inv_freq = (1.0 / (ROPE_BASE ** (np.arange(0, HD, 2, dtype=np.float32) / HD))
                ).astype(np.float32)
    pos = trows.astype(np.float32)
    freqs = pos[:, None] * inv_freq[None, :]
    emb = np.concatenate([freqs, freqs], axis=-1)[:, ::2]      # (n, 32)
    return np.cos(emb).astype(np.float32), np.sin(emb).astype(np.float32)


def _allowed(i, j):
    ok = (np.abs(i - j) <= W) | (j % GSTRIDE == 0) | (j == 0)
    return ok & (j >= 0) & (j < T)


NEG = np.float16(-30000.0)


def make_in_maps(x, in_proj_w, in_proj_b, out_w, out_b):
    perm = _perm()
    x2 = np.asarray(x, np.float32).reshape(T, D)[:, perm]
    wp = np.asarray(in_proj_w, np.float32)[:, perm]
    wt_full = np.ascontiguousarray(wp.T).astype(np.float32)     # (D, 3D)
    wt_full[:, 0:D] *= SCALE
    b = np.asarray(in_proj_b, np.float32).copy()
    b[0:D] *= SCALE
    ball = np.ascontiguousarray(b.reshape(24, 128).T).astype(np.float16)
    bvp = np.ascontiguousarray(b[2 * D:][None, :]).astype(np.float16)
    wo_full = np.ascontiguousarray(np.asarray(out_w, np.float32).T)  # (din,dout)
    # wall[p, i, :]: i<24: wt_full[128*(i%8)+p, (i//8)*D:(i//8+1)*D];
    # i>=24: wo_full[128*(i-24)+p, :]
    wall = np.empty((128, 32, D), np.float32)
    for s in range(3):
        for k in range(NDT):
            wall[:, 8 * s + k, :] = wt_full[128 * k:128 * (k + 1),
                                            s * D:(s + 1) * D]
    for k in range(NDT):
        wall[:, 24 + k, :] = wo_full[128 * k:128 * (k + 1), :]
    wall = np.ascontiguousarray(wall.reshape(128, 32 * D)).astype(np.float16)
    bo = np.ascontiguousarray(np.asarray(out_b, np.float32)[None, :]
                              ).astype(np.float16)

    tg = np.arange(NG) * GSTRIDE
    cg, sg = _cos_sin(tg)
    csg = np.concatenate([cg, sg], axis=1)

    sel8 = np.zeros((8, 4, 128), np.float16)
    for v in range(4):
        sel8[2 * v, v, 0:64] = 1
        sel8[2 * v + 1, v, 64:128] = 1
    sel8 = np.ascontiguousarray(sel8.reshape(8, 512))

    ident = np.eye(128, dtype=np.float16)

    in_maps = []
    for c in range(NCORES):
        t0 = c * TLOC - HALO
        rows = np.arange(t0, t0 + NL)
        valid = (rows >= 0) & (rows < T)
        xcsc = np.zeros((NR, D + 64), np.float32)
        xcsc[:NL, 0:D][valid] = x2[rows[valid]]
        xcsc[NL:, 0:D] = x2[tg]
        cl, sl = _cos_sin(np.clip(rows, 0, T - 1))
        xcsc[:, D:] = np.concatenate(
            [np.concatenate([cl, sl], axis=1), csg], axis=0)

        ml = np.full((NT, 128, 384), NEG, np.float16)
        for m in range(NT):
            jj = (t0 + m * 128) + np.arange(128)
            ii = c * TLOC + QS[m] + np.arange(QW[m])
            ml[m, :, 0:QW[m]] = np.where(
                _allowed(ii[None, :], jj[:, None]), np.float16(0), NEG)
        mloc = np.ascontiguousarray(ml.transpose(1, 0, 2).reshape(128, NT * 384))
        iq = c * TLOC + np.arange(TLOC)
        qb = iq // 128
        jg = tg[:, None]
        covered = (jg >= 128 * (qb[None, :] - 1)) & (jg < 128 * (qb[None, :] + 2))
        mg1 = np.where(covered, NEG, np.float16(0)).astype(np.float16)
        mgp = np.full((48, TLOC), NEG, np.float16)
        mgp[0:16] = mg1
        mgp[32:48] = mg1

        in_maps.append({
            "xcs": xcsc.astype(np.float16), "identd": ident, "wall": wall,
            "ball": ball, "bvp": bvp, "bo": bo, "mloc": mloc, "mgp": mgp,
            "sel8": sel8,
        })
    return in_maps


def _patch_ldw_opt():
    """Enable walrus LDWEIGHTS overlap (pull-ahead) — the concourse default
    pins --enable-ldw-opt=false; with per-MM weight reloads that serializes
    ~100ns of weight-load into every matmul."""
    import os
    if os.environ.get("KLDW", "0") != "1":
        return
    import concourse.bass_utils as _bu
    if getattr(_bu, "_ldw_opt_patched", False):
        return
    _orig = _bu.run_command

    def _rc(argv, **kw):
        argv = ["--enable-ldw-opt=true" if a == "--enable-ldw-opt=false"
                else a for a in argv]
        return _orig(argv, **kw)

    _bu.run_command = _rc
    _bu._ldw_opt_patched = True


def kernel(x, in_proj_w, in_proj_b, out_w, out_b):
    from concourse.bass_utils import run_bass_kernel_spmd

    _patch_ldw_opt()

    has_vbias = bool(np.any(np.asarray(in_proj_b)[2 * D:] != 0))
    has_obias = bool(np.any(np.asarray(out_b) != 0))
    key = ("nc", has_vbias, has_obias)
    if key not in _cache:
        _cache[key] = build_program(has_vbias, has_obias)
        _cache["nc"] = _cache[key]
    nc = _cache[key]
    in_maps = make_in_maps(x, in_proj_w, in_proj_b, out_w, out_b)
    res = run_bass_kernel_spmd(nc, in_maps, list(range(NCORES))).results
    pieces = [res[c]["out"] for c in range(NCORES)]
    return np.concatenate(pieces, axis=0).reshape(B, T, D).astype(np.float32)
